# revision 12
# baseline (speedup 1.0000x reference)
"""MCWAUCHLoss Trainium2 kernel.

Shards the [B, C] = [65536, 256] inputs row-wise across 8 NeuronCores
(8192 rows each). Inputs are cast to bf16 on host (labels exactly
representable; x rounding washes out across the >=8k-element reductions).

Per core, per tile (phase A = sigmoid table set, phase B = natural_log):
  A:  s    = sigmoid(x)                (ACT)
      labc = 1 - lab                   (DVE tensor_scalar)
      lt   = lab * s                   (DVE)
      w1   = lt + labc                 (DVE)   -> s where lab=1 else 1 (exact)
      d    = s - lt                    (DVE)   -> s where lab=0 else 0 (exact)
      PSUM s  += ones^T @ s            (PE, per-category)
      PSUM lt += ones^T @ lt           (PE, per-category)
  B:  acc_pl[t] = sum ln(w1)           (ACT accum_out)  = sum lab*ln(s)
      acc_nl[t] = sum ln(1 - d)        (ACT accum_out, scale=-1 bias=1)
                                       = sum (1-lab)*ln(1-s)
ln(1) = 0 makes the masking exact. n_pos comes from a host-side
labels.sum(0); no x-only reductions are needed on device.
"""

import sys

import numpy as np

sys.path.insert(0, "/opt/trn_rl_repo")

from contextlib import ExitStack


def _ensure_axon_hooks():
    """Provide antenv.axon_hooks if the image lacks it (needed only when
    profiling with trace=True; harmless otherwise)."""
    try:
        import antenv.axon_hooks  # noqa: F401
        return
    except ImportError:
        pass
    import types

    try:
        import antenv
    except ImportError:
        return
    mod = types.ModuleType("antenv.axon_hooks")
    mod._HOOK = None

    def set_axon_ntff_profile_hook(h):
        mod._HOOK = h

    def get_axon_ntff_profile_hook():
        if mod._HOOK is None:
            try:
                from trn_agent_boot.trn_boot import _ntff_profile_via_ctypes

                mod._HOOK = _ntff_profile_via_ctypes("/opt/axon/libaxon_pjrt.so")
            except Exception:
                return None
        return mod._HOOK

    mod.set_axon_ntff_profile_hook = set_axon_ntff_profile_hook
    mod.get_axon_ntff_profile_hook = get_axon_ntff_profile_hook
    sys.modules["antenv.axon_hooks"] = mod
    antenv.axon_hooks = mod


_ensure_axon_hooks()

import ml_dtypes
import concourse.bacc as bacc
import concourse.tile as tile
from concourse import mybir
from concourse.tile import add_dep_helper
from concourse.bass_utils import run_bass_kernel_spmd

B, C = 65536, 256
N_CORES = 8
R = B // N_CORES            # 8192 rows per core
TILE_ROWS = 2048            # rows per SBUF tile
T = R // TILE_ROWS          # 4 tiles per core
P = 128                     # partitions
RG = TILE_ROWS // P         # 16 rowgroups per tile
FREE = RG * C               # 4096 free elements per partition
MM_N = 512                  # matmul moving free dim (2 rowgroups worth)
MM_PER_TILE = FREE // MM_N  # 8

BF = mybir.dt.bfloat16
F32 = mybir.dt.float32

_PROGRAM = None


def _build_program():
    nc = bacc.Bacc("TRN2", target_bir_lowering=False, debug=False)

    x_d = nc.dram_tensor("x", [R, C], BF, kind="ExternalInput").ap()
    lab_d = nc.dram_tensor("lab", [R, C], BF, kind="ExternalInput").ap()
    # rows: 0 = sum s, 1 = sum lab*s   (col j: category j%256, even/odd
    # rowgroup half j//256)
    o_cat = nc.dram_tensor("o_cat", [2, MM_N], F32, kind="ExternalOutput").ap()
    # rows: 0 = sum ln(w1) = PL part, 1 = sum ln(1-d) = NL part
    o_acc = nc.dram_tensor("o_acc", [2, P, T], F32, kind="ExternalOutput").ap()

    with tile.TileContext(nc) as tc, ExitStack() as ctx:
        const = ctx.enter_context(tc.tile_pool(name="const", bufs=1))
        xp = ctx.enter_context(tc.tile_pool(name="xp", bufs=1))
        labp = ctx.enter_context(tc.tile_pool(name="labp", bufs=1))
        sp = ctx.enter_context(tc.tile_pool(name="sp", bufs=2))
        wp = ctx.enter_context(tc.tile_pool(name="wp", bufs=1))
        work = ctx.enter_context(tc.tile_pool(name="work", bufs=2))
        workc = ctx.enter_context(tc.tile_pool(name="workc", bufs=1))
        accp = ctx.enter_context(tc.tile_pool(name="accp", bufs=1))
        psum = ctx.enter_context(tc.tile_pool(name="psum", bufs=1, space="PSUM"))

        ones = const.tile([P, 1], BF, tag="ones")
        nc.vector.memset(ones, 1.0)

        acc_pl = accp.tile([P, T], F32, tag="acc_pl")
        acc_nl = accp.tile([P, T], F32, tag="acc_nl")

        ps_s = psum.tile([1, MM_N], F32, tag="ps_s")
        ps_lt = psum.tile([1, MM_N], F32, tag="ps_lt")


        mul = mybir.AluOpType.mult
        add = mybir.AluOpType.add
        sub = mybir.AluOpType.subtract

        # --- input DMAs: all x tiles first (sigmoid is the critical path),
        # lab tiles after ---
        xts = []
        labs = []
        for t in range(T):
            rows = slice(t * TILE_ROWS, (t + 1) * TILE_ROWS)
            xt = xp.tile([P, FREE], BF, tag=f"x{t}")
            nc.sync.dma_start(
                out=xt, in_=x_d[rows, :].rearrange("(p r) c -> p (r c)", p=P)
            )
            xts.append(xt)
        for t in range(T):
            rows = slice(t * TILE_ROWS, (t + 1) * TILE_ROWS)
            lab = labp.tile([P, FREE], BF, tag=f"lab{t}")
            nc.sync.dma_start(
                out=lab, in_=lab_d[rows, :].rearrange("(p r) c -> p (r c)", p=P)
            )
            labs.append(lab)

        # --- phase A: sigmoid table set + products + PE reductions ---
        acts_a = []
        w1ft = []
        dt_ = []
        for t in range(T):
            xt = xts[t]
            lab = labs[t]
            s = sp.tile([P, FREE], BF, tag="s")
            ia = nc.scalar.activation(
                out=s, in_=xt, func=mybir.ActivationFunctionType.Sigmoid
            )
            acts_a.append(ia)

            # labc = 1 - lab on the (otherwise idle) Pool engine
            labc = workc.tile([P, FREE], BF, tag="labc")
            nc.gpsimd.tensor_scalar(
                out=labc, in0=lab, scalar1=-1.0, scalar2=1.0, op0=mul, op1=add
            )
            lt = work.tile([P, FREE], BF, tag="lt")
            nc.vector.tensor_mul(lt, lab, s)
            w1 = workc.tile([P, FREE], BF, tag="w1")
            nc.vector.tensor_tensor(out=w1, in0=lt, in1=labc, op=add)
            # log-fold: ln(a) + ln(b) = ln(a*b) -> halve the Ln pass elements
            w1f = wp.tile([P, FREE // 2], BF, tag=f"w1f_{t}")
            nc.vector.tensor_mul(w1f, w1[:, : FREE // 2], w1[:, FREE // 2 :])
            d = wp.tile([P, FREE], BF, tag=f"d_{t}")
            nc.vector.tensor_tensor(out=d, in0=s, in1=lt, op=sub)
            w1ft.append(w1f)
            dt_.append(d)

            for k in range(MM_PER_TILE):
                first = t == 0 and k == 0
                last = t == T - 1 and k == MM_PER_TILE - 1
                sl = slice(k * MM_N, (k + 1) * MM_N)
                nc.tensor.matmul(ps_s, ones, s[:, sl], start=first, stop=last)
                nc.tensor.matmul(ps_lt, ones, lt[:, sl], start=first, stop=last)

        # --- phase B: natural_log table set, accumulating scalar sums ---
        acts_b = []
        for t in range(T):
            ib = nc.scalar.activation(
                out=w1ft[t],
                in_=w1ft[t],
                func=mybir.ActivationFunctionType.Ln,
                accum_out=acc_pl[:, t : t + 1],
            )
            acts_b.append(ib)
            ib2 = nc.scalar.activation(
                out=dt_[t],
                in_=dt_[t],
                func=mybir.ActivationFunctionType.Ln,
                scale=-1.0,
                bias=1.0,
                accum_out=acc_nl[:, t : t + 1],
            )
            acts_b.append(ib2)

        # keep the ACT engine phase-ordered: each table set loads exactly once
        for ia in acts_a:
            for ib in acts_b:
                # first arg waits on second: every Ln runs after every Sigmoid
                add_dep_helper(
                    ib.ins, ia.ins, sync=False, reason="act table phase order"
                )

        # --- outputs (PSUM staged through SBUF; engine writes must start
        # at partition 0, so one [1, N] tile per quantity) ---
        for i, ps in enumerate([ps_s, ps_lt]):
            cat_sb = accp.tile([1, MM_N], F32, tag=f"cat_sb{i}")
            nc.vector.tensor_copy(cat_sb, ps)
            nc.sync.dma_start(out=o_cat[i : i + 1, :], in_=cat_sb)
        nc.sync.dma_start(out=o_acc[0], in_=acc_pl)
        nc.sync.dma_start(out=o_acc[1], in_=acc_nl)

    nc.compile()
    return nc


def _get_program():
    global _PROGRAM
    if _PROGRAM is None:
        _PROGRAM = _build_program()
    return _PROGRAM


def _run_on_hw(x, lab, **kwargs):
    nc = _get_program()
    xb = np.asarray(x, dtype=np.float32).astype(ml_dtypes.bfloat16)
    lb = np.asarray(lab, dtype=np.float32).astype(ml_dtypes.bfloat16)
    in_maps = []
    for m in range(N_CORES):
        rows = slice(m * R, (m + 1) * R)
        in_maps.append(
            {
                "x": np.ascontiguousarray(xb[rows]),
                "lab": np.ascontiguousarray(lb[rows]),
            }
        )
    return run_bass_kernel_spmd(nc, in_maps, core_ids=list(range(N_CORES)), **kwargs)


def _combine(results, labels):
    sum_s = np.zeros(C, np.float64)
    sum_pos = np.zeros(C, np.float64)
    PL = 0.0
    NL = 0.0
    for r in results:
        cat = r["o_cat"].astype(np.float64)
        sum_s += cat[0, :C] + cat[0, C:]
        sum_pos += cat[1, :C] + cat[1, C:]
        acc = r["o_acc"].astype(np.float64)
        PL += acc[0].sum()
        NL += acc[1].sum()

    n_pos = labels.sum(axis=0, dtype=np.float64)
    total = float(B) * float(C)
    num_P = n_pos.sum()
    alpha_P = num_P / total
    alpha_N = (total - num_P) / total
    cel = -alpha_N * (PL / total) - alpha_P * (NL / total)

    n_neg = float(B) - n_pos
    mean_pos = sum_pos / np.maximum(n_pos, 1.0)
    mean_neg = (sum_s - sum_pos) / np.maximum(n_neg, 1.0)
    both = (n_pos > 0) & (n_neg > 0)
    pen = np.where(
        both,
        1.0 - mean_pos + mean_neg,
        np.where(n_pos == 0, 1.0 + mean_neg, 1.0 - mean_pos),
    )
    cls = cel + 0.1 * (pen.sum() / C)
    return (np.float32(cls), np.float32(0.1 * pen[-1]))


def kernel(output, labels):
    res = _run_on_hw(output, labels)
    return _combine(res.results, np.asarray(labels))


if __name__ == "__main__":
    x = np.random.randn(B, C).astype(np.float32)
    lab = (np.random.rand(B, C) < 0.3).astype(np.float32)
    print(kernel(output=x, labels=lab))


# revision 14
# speedup vs baseline: 1.2077x; 1.2077x over previous
"""MCWAUCHLoss Trainium2 kernel.

Shards the [B, C] = [65536, 256] inputs row-wise across 8 NeuronCores
(8192 rows each). Inputs are cast to bf16 on host (labels exactly
representable; x rounding washes out across the >=8k-element reductions).

Per core, per tile (phase A = sigmoid table set, phase B = natural_log):
  A:  s    = sigmoid(x)                (ACT)
      labc = 1 - lab                   (DVE tensor_scalar)
      lt   = lab * s                   (DVE)
      w1   = lt + labc                 (DVE)   -> s where lab=1 else 1 (exact)
      d    = s - lt                    (DVE)   -> s where lab=0 else 0 (exact)
      PSUM s  += ones^T @ s            (PE, per-category)
      PSUM lt += ones^T @ lt           (PE, per-category)
  B:  acc_pl[t] = sum ln(w1)           (ACT accum_out)  = sum lab*ln(s)
      acc_nl[t] = sum ln(1 - d)        (ACT accum_out, scale=-1 bias=1)
                                       = sum (1-lab)*ln(1-s)
ln(1) = 0 makes the masking exact. n_pos comes from a host-side
labels.sum(0); no x-only reductions are needed on device.
"""

import sys

import numpy as np

sys.path.insert(0, "/opt/trn_rl_repo")

from contextlib import ExitStack


def _ensure_axon_hooks():
    """Provide antenv.axon_hooks if the image lacks it (needed only when
    profiling with trace=True; harmless otherwise)."""
    try:
        import antenv.axon_hooks  # noqa: F401
        return
    except ImportError:
        pass
    import types

    try:
        import antenv
    except ImportError:
        return
    mod = types.ModuleType("antenv.axon_hooks")
    mod._HOOK = None

    def set_axon_ntff_profile_hook(h):
        mod._HOOK = h

    def get_axon_ntff_profile_hook():
        if mod._HOOK is None:
            try:
                from trn_agent_boot.trn_boot import _ntff_profile_via_ctypes

                mod._HOOK = _ntff_profile_via_ctypes("/opt/axon/libaxon_pjrt.so")
            except Exception:
                return None
        return mod._HOOK

    mod.set_axon_ntff_profile_hook = set_axon_ntff_profile_hook
    mod.get_axon_ntff_profile_hook = get_axon_ntff_profile_hook
    sys.modules["antenv.axon_hooks"] = mod
    antenv.axon_hooks = mod


_ensure_axon_hooks()

import ml_dtypes
import concourse.bacc as bacc
import concourse.tile as tile
from concourse import mybir
from concourse.tile import add_dep_helper
from concourse.bass_utils import run_bass_kernel_spmd

B, C = 65536, 256
N_CORES = 8
R = B // N_CORES            # 8192 rows per core
TILE_ROWS = 2048            # rows per SBUF tile
T = R // TILE_ROWS          # 4 tiles per core
P = 128                     # partitions
RG = TILE_ROWS // P         # 16 rowgroups per tile
FREE = RG * C               # 4096 free elements per partition
MM_N = 512                  # matmul moving free dim (2 rowgroups worth)
MM_PER_TILE = FREE // MM_N  # 8

BF = mybir.dt.bfloat16
F32 = mybir.dt.float32

_PROGRAM = None


def _build_program():
    nc = bacc.Bacc("TRN2", target_bir_lowering=False, debug=False)

    x_d = nc.dram_tensor("x", [R, C], BF, kind="ExternalInput").ap()
    lab_d = nc.dram_tensor("lab", [R, C], BF, kind="ExternalInput").ap()
    # rows: 0 = sum s, 1 = sum lab*s   (col j: category j%256, even/odd
    # rowgroup half j//256)
    o_cat = nc.dram_tensor("o_cat", [2, MM_N], F32, kind="ExternalOutput").ap()
    # rows: 0 = sum ln(w1) = PL part, 1 = sum ln(1-d) = NL part
    o_acc = nc.dram_tensor("o_acc", [2, P, T], F32, kind="ExternalOutput").ap()

    with tile.TileContext(nc) as tc, ExitStack() as ctx:
        const = ctx.enter_context(tc.tile_pool(name="const", bufs=1))
        xp = ctx.enter_context(tc.tile_pool(name="xp", bufs=1))
        labp = ctx.enter_context(tc.tile_pool(name="labp", bufs=1))
        sp = ctx.enter_context(tc.tile_pool(name="sp", bufs=3))
        wp = ctx.enter_context(tc.tile_pool(name="wp", bufs=1))
        work = ctx.enter_context(tc.tile_pool(name="work", bufs=2))
        workc = ctx.enter_context(tc.tile_pool(name="workc", bufs=1))
        accp = ctx.enter_context(tc.tile_pool(name="accp", bufs=1))
        psum = ctx.enter_context(tc.tile_pool(name="psum", bufs=1, space="PSUM"))

        ones = const.tile([P, 1], BF, tag="ones")
        nc.vector.memset(ones, 1.0)

        acc_pl = accp.tile([P, T], F32, tag="acc_pl")
        acc_nl = accp.tile([P, T], F32, tag="acc_nl")

        ps_s = psum.tile([1, MM_N], F32, tag="ps_s")
        ps_lt = psum.tile([1, MM_N], F32, tag="ps_lt")


        mul = mybir.AluOpType.mult
        add = mybir.AluOpType.add
        sub = mybir.AluOpType.subtract

        # --- input DMAs: interleave x/lab per tile so the DVE product
        # chain starts as early as the first sigmoid ---
        xts = []
        labs = []
        for t in range(T):
            rows = slice(t * TILE_ROWS, (t + 1) * TILE_ROWS)
            xt = xp.tile([P, FREE], BF, tag=f"x{t}")
            nc.sync.dma_start(
                out=xt, in_=x_d[rows, :].rearrange("(p r) c -> p (r c)", p=P)
            )
            xts.append(xt)
            lab = labp.tile([P, FREE], BF, tag=f"lab{t}")
            nc.sync.dma_start(
                out=lab, in_=lab_d[rows, :].rearrange("(p r) c -> p (r c)", p=P)
            )
            labs.append(lab)

        # --- phase A: sigmoid table set + products + PE reductions ---
        acts_a = []
        w1t = []
        w1ft = []
        dt_ = []
        for t in range(T):
            xt = xts[t]
            lab = labs[t]
            s = sp.tile([P, FREE], BF, tag="s")
            ia = nc.scalar.activation(
                out=s, in_=xt, func=mybir.ActivationFunctionType.Sigmoid
            )
            acts_a.append(ia)

            lt = work.tile([P, FREE], BF, tag="lt")
            nc.vector.tensor_mul(lt, lab, s)
            labc = work.tile([P, FREE], BF, tag="labc")
            nc.vector.tensor_scalar(
                out=labc, in0=lab, scalar1=-1.0, scalar2=1.0, op0=mul, op1=add
            )
            w1 = workc.tile([P, FREE], BF, tag=f"w1_{t % 2}")
            nc.vector.tensor_tensor(out=w1, in0=lt, in1=labc, op=add)
            w1t.append(w1)
            d = wp.tile([P, FREE], BF, tag=f"d_{t}")
            nc.vector.tensor_tensor(out=d, in0=s, in1=lt, op=sub)
            dt_.append(d)
            if t % 2 == 1:
                # log-fold across the tile pair: ln(a)+ln(b) = ln(a*b);
                # distinct tensors keep the DVE 2x perf mode
                w1f = wp.tile([P, FREE], BF, tag=f"w1f_{t // 2}")
                nc.vector.tensor_mul(w1f, w1t[t - 1], w1t[t])
                w1ft.append(w1f)

            for k in range(MM_PER_TILE):
                first = t == 0 and k == 0
                last = t == T - 1 and k == MM_PER_TILE - 1
                sl = slice(k * MM_N, (k + 1) * MM_N)
                nc.tensor.matmul(ps_s, ones, s[:, sl], start=first, stop=last)
                nc.tensor.matmul(ps_lt, ones, lt[:, sl], start=first, stop=last)

        # --- phase B: natural_log table set, accumulating scalar sums ---
        acts_b = []
        for i, w1f in enumerate(w1ft):
            ib = nc.scalar.activation(
                out=w1f,
                in_=w1f,
                func=mybir.ActivationFunctionType.Ln,
                accum_out=acc_pl[:, i : i + 1],
            )
            acts_b.append(ib)
        for t in range(T):
            ib2 = nc.scalar.activation(
                out=dt_[t],
                in_=dt_[t],
                func=mybir.ActivationFunctionType.Ln,
                scale=-1.0,
                bias=1.0,
                accum_out=acc_nl[:, t : t + 1],
            )
            acts_b.append(ib2)

        # keep the ACT engine phase-ordered: each table set loads exactly once
        for ia in acts_a:
            for ib in acts_b:
                # first arg waits on second: every Ln runs after every Sigmoid
                add_dep_helper(
                    ib.ins, ia.ins, sync=False, reason="act table phase order"
                )

        # --- outputs (PSUM staged through SBUF; engine writes must start
        # at partition 0, so one [1, N] tile per quantity) ---
        for i, ps in enumerate([ps_s, ps_lt]):
            cat_sb = accp.tile([1, MM_N], F32, tag=f"cat_sb{i}")
            nc.vector.tensor_copy(cat_sb, ps)
            nc.sync.dma_start(out=o_cat[i : i + 1, :], in_=cat_sb)
        nc.sync.dma_start(out=o_acc[0], in_=acc_pl)
        nc.sync.dma_start(out=o_acc[1], in_=acc_nl)

    nc.compile()
    return nc


def _get_program():
    global _PROGRAM
    if _PROGRAM is None:
        _PROGRAM = _build_program()
    return _PROGRAM


def _run_on_hw(x, lab, **kwargs):
    nc = _get_program()
    xb = np.asarray(x, dtype=np.float32).astype(ml_dtypes.bfloat16)
    lb = np.asarray(lab, dtype=np.float32).astype(ml_dtypes.bfloat16)
    in_maps = []
    for m in range(N_CORES):
        rows = slice(m * R, (m + 1) * R)
        in_maps.append(
            {
                "x": np.ascontiguousarray(xb[rows]),
                "lab": np.ascontiguousarray(lb[rows]),
            }
        )
    return run_bass_kernel_spmd(nc, in_maps, core_ids=list(range(N_CORES)), **kwargs)


def _combine(results, labels):
    sum_s = np.zeros(C, np.float64)
    sum_pos = np.zeros(C, np.float64)
    PL = 0.0
    NL = 0.0
    for r in results:
        cat = r["o_cat"].astype(np.float64)
        sum_s += cat[0, :C] + cat[0, C:]
        sum_pos += cat[1, :C] + cat[1, C:]
        acc = r["o_acc"].astype(np.float64)
        PL += acc[0].sum()
        NL += acc[1].sum()

    n_pos = labels.sum(axis=0, dtype=np.float64)
    total = float(B) * float(C)
    num_P = n_pos.sum()
    alpha_P = num_P / total
    alpha_N = (total - num_P) / total
    cel = -alpha_N * (PL / total) - alpha_P * (NL / total)

    n_neg = float(B) - n_pos
    mean_pos = sum_pos / np.maximum(n_pos, 1.0)
    mean_neg = (sum_s - sum_pos) / np.maximum(n_neg, 1.0)
    both = (n_pos > 0) & (n_neg > 0)
    pen = np.where(
        both,
        1.0 - mean_pos + mean_neg,
        np.where(n_pos == 0, 1.0 + mean_neg, 1.0 - mean_pos),
    )
    cls = cel + 0.1 * (pen.sum() / C)
    return (np.float32(cls), np.float32(0.1 * pen[-1]))


def kernel(output, labels):
    res = _run_on_hw(output, labels)
    return _combine(res.results, np.asarray(labels))


if __name__ == "__main__":
    x = np.random.randn(B, C).astype(np.float32)
    lab = (np.random.rand(B, C) < 0.3).astype(np.float32)
    print(kernel(output=x, labels=lab))
